# revision 39
# baseline (speedup 1.0000x reference)
"""Trainium2 Bass kernel for nn_ExpertRouter (dense MoE routing).

Reference computation (per token t of T=4096, D=6144, MID=512, NE=16):
    h[t,n,:] = relu(xf[t] @ w1[n] + b1[n])          # [T, NE, MID]
    e[t,n]   = h[t,n] . w2[n] + b2[n]               # [T, NE]
    g[t,:]   = softmax(xf[t] @ gw + gb)             # [T, NE]
    out[t]   = sigmoid(sum_n g[t,n] * e[t,n])

Strategy: data-parallel over tokens across 8 NeuronCores (512 tokens/core,
weights replicated, no collectives). Dominant compute = 16 expert matmuls
[512,6144]@[6144,512] per core in fp8-e4m3 DoubleRowSwInterleave mode.
Trace-verified: the w1 matmul stream runs back-to-back at the 518-cycle
(216 ns) floor for N=512, so the kernel is PE-streaming-bound; all
optimization beyond that targets head/tail latency and pass-count:

- PE warmup: ~200 short (N=128) dummy matmuls with zero DMA deps run
  during the NEFF preamble + head DMA wait and un-throttle the HAM
  clock (first ~27 run at half clock, then full speed), so the real
  stream starts warm. Fine granularity bounds overshoot to ~100 ns.
- DMA queues: gw + xq chunks on sync, w1 chunks on gpsimd, small
  consts + out on scalar. Each dma_start costs ~0.65 us of
  issuing-queue time (DGE descriptor gen), so chunk counts are kept
  low; the first xq chunks are small (2,4,6,6,6 k2-steps) because all
  8 cores burst-DMA simultaneously at the head.
- Gating is interleaved into expert-0-mt0 per k2-step (one open psum
  group each in separate banks) so the head consumes xq as it lands.
- All 32 e-dot passes run back-to-back at the stream tail, where they
  hit the 216 ns floor (inline they measured 379 ns) and form ONE psum
  accumulation group into e_ps_all[32, TOK]: expert n's zero-padded w2
  stationary has its live column at position n, so its scalar lands on
  psum row n and the padding adds 0 to other rows - no gather needed.
  All 32 h2 pair tiles stay live (hbufs=32, ~32KB/partition).
- Epilogue: m = (e_ps_all[0:16] + 1024*b2) * expl_norm via one
  scalar_tensor_tensor (bf16), u = ones16^T @ m (one pass), then Tanh:
  sigmoid's ACT table-set differs from exp/relu's (a ~2.7 us reload on
  the tail), but tanh shares exp's set; sigmoid(z)=0.5+0.5*tanh(z/2)
  and the affine is applied on host during unsharding. Softmax
  normalization happens early: reciprocal + stream_shuffle partition
  broadcast + one [16,TOK] mul on DVE, hidden under the e-dots.

HW notes (measured on trn2, do not trust CoreSim for these):
- fp8 DoubleRow is 2x bf16; 512-col pass floor 216 ns back-to-back;
  SwInterleave beats plain DoubleRow ~4-6%/pass; repeated identical
  stationaries do NOT skip Ldweights.
- walrus codegen crashes on DoubleRow/SwInterleave with narrow
  stationaries (1-wide DR, 16-wide SWI, 32-wide SWI) - hence 32-wide
  zero-padded DR w2 and plain-DR gating.
- a +163 ns hiccup hits one pass every ~10.8 us of PE streaming
  (~31/rep, ~5 us) regardless of which pass sits there; unexplained
  (HAM/PSUM micro-throttle?), unfixed.
- sustained (100+ ms) execution downclocks the PE (P0 power state):
  4-rep trace period ~365 us vs ~443-492 ns/rep marginal cost at
  128-512 reps. Micro-optimizations drown in this on sustained
  metrics; only adjacent paired A/B comparisons are trustworthy
  (~+-2 us/rep resolution at n=32).
- single-exec trace (this config): MM span ~364 us, total PE gap
  ~4.8 us, exec_time ~377 us; v1 baseline was ~380/373.
"""

import contextlib
import numpy as np
import ml_dtypes

# problem constants (hardcoded per harness contract)
B, NW, WS, FD = 16, 256, 8, 96
D = WS * WS * FD          # 6144
MID = 512
NE = 16
T = B * NW                # 4096 tokens
NCORES = 8
TOK = T // NCORES         # 512 tokens per core
P = 128                   # partitions
KT = D // P               # 48 contraction tiles
KT2 = KT // 2             # 24 DoubleRow k-steps (256 contraction per pass)
MT = MID // P             # 4 mid tiles
X_SCALE = 128.0           # w1/gw pre-scale: U(-1/sqrt(D),..) -> e4m3 normal range
H_SCALE = 16.0            # h pre-scale into e4m3 (h in [0,~4])
W2_SCALE = 64.0           # w2 pre-scale: U(-1/sqrt(MID),..) -> e4m3 normal range
E_SCALE = H_SCALE * W2_SCALE  # net scale on e_ps; folded into b2 + final sigmoid
E_SWI = False              # e-dot stationaries in SwInterleave layout

_CACHE = {}


def _build(reps=1, wbufs=3, xbufs=2, ps_hbufs=4, hbufs=32, use_swi=True,
           warm=200, interleave=True, swi_e=E_SWI, tanh_out=True,
           xsched=(2, 4, 6, 6, 6)):
    """Build + compile the per-core SPMD bass program. Returns nc.

    reps>1 wraps the whole body in a Tile For loop - used only for
    slope-based HW timing (fixed dispatch overhead cancels between rep
    counts); the graded kernel uses reps=1 (no loop)."""
    import concourse.tile as tile
    from concourse import bacc, mybir

    fp8 = mybir.dt.float8e4
    bf16 = mybir.dt.bfloat16
    f32 = mybir.dt.float32
    AF = mybir.ActivationFunctionType
    ALU = mybir.AluOpType
    SWI = (mybir.MatmulPerfMode.DoubleRowSwInterleave if use_swi
           else mybir.MatmulPerfMode.DoubleRow)
    DR = mybir.MatmulPerfMode.DoubleRow

    nc = bacc.Bacc("TRN2", target_bir_lowering=False, debug=False)

    xq_d = nc.dram_tensor("xq", [P, KT2, 2, TOK], fp8, kind="ExternalInput").ap()
    # mt-major chunks so each [P, KT2, P, 2] mid-chunk is one contiguous DMA;
    # last two dims are the SwInterleave layout (reversed mid, row-pair minor)
    w1_d = nc.dram_tensor(
        "w1", [NE, MT, P, KT2, P, 2], fp8, kind="ExternalInput"
    ).ap()
    gw_d = nc.dram_tensor("gw", [P, KT2, 2, NE], fp8, kind="ExternalInput").ap()
    b1_d = nc.dram_tensor("b1", [P, NE, MT], f32, kind="ExternalInput").ap()
    # w2 stationaries: expert n's live column at psum row n; SWI layout
    # [Ki, 32, 2] (cols reversed, pair minor) or DR layout [Ki, 2, 32]
    w2_shape = [P, NE, 2, 32, 2] if swi_e else [P, NE, 2, 2, 32]
    w2_d = nc.dram_tensor("w2", w2_shape, fp8, kind="ExternalInput").ap()
    b2_d = nc.dram_tensor("b2", [NE, 1], f32, kind="ExternalInput").ap()
    gb_d = nc.dram_tensor("gb", [NE, 1], f32, kind="ExternalInput").ap()
    out_d = nc.dram_tensor("out", [1, TOK], f32, kind="ExternalOutput").ap()

    with tile.TileContext(nc) as tc:
        # PE warmup, outside the rep loop: dummy matmuls with no DMA deps
        # run during the NEFF preamble / first DMA wait and un-throttle the
        # HAM clock before the real stream starts. Pools released before
        # the main pools open so their space is reused.
        if warm:
            with (
                tc.tile_pool(name="wu", bufs=1) as wup,
                tc.tile_pool(name="wups", bufs=1, space="PSUM") as wups,
            ):
                wmov = wup.tile([P, TOK], fp8)
                nc.vector.memset(wmov[:], 0.0)
                wps = wups.tile([P, P], f32)
                # short N=128 passes: fine-grained, so if the head DMAs land
                # early the overshoot is at most ~one pass (~100 ns)
                for _ in range(warm):
                    nc.tensor.matmul(
                        wps[:], wmov[:, 0:P], wmov[:, 0:P], start=True,
                        stop=True, skip_group_check=True,
                    )

        loop_ctx = (
            tc.For_i(0, reps, 1) if reps > 1 else contextlib.nullcontext()
        )
        with (
            loop_ctx,
            tc.tile_pool(name="consts", bufs=1) as consts,
            tc.tile_pool(name="consts2", bufs=1) as consts2,
            tc.tile_pool(name="xpool", bufs=xbufs) as xpool,
            tc.tile_pool(name="wpool", bufs=wbufs) as wpool,
            tc.tile_pool(name="hpool", bufs=hbufs) as hpool,
            tc.tile_pool(name="small", bufs=2) as small,
            tc.tile_pool(name="ps_h", bufs=ps_hbufs, space="PSUM") as ps_h,
            tc.tile_pool(name="ps_g", bufs=1, space="PSUM") as ps_g,
            tc.tile_pool(name="ps_e", bufs=1, space="PSUM") as ps_e,
            tc.tile_pool(name="ps_u", bufs=1, space="PSUM") as ps_u,
        ):
            # head DMAs: gw + resident x (chunked) on the sync queue. First
            # chunks are small so the gating chain starts ASAP (all 8 cores
            # burst-DMA simultaneously at the head, so early bytes are
            # precious); later chunks are big (each dma_start costs ~0.65us
            # of issuing-queue time).
            gw = consts.tile([P, KT2, 2, NE], fp8)
            nc.sync.dma_start(gw[:], gw_d[:])
            xq = xpool.tile([P, KT2, 2, TOK], fp8)
            k0 = 0
            for kch in xsched:
                nc.sync.dma_start(
                    xq[:, k0:k0 + kch, :, :], xq_d[:, k0:k0 + kch, :, :]
                )
                k0 += kch
            assert k0 == KT2
            # small consts on the scalar queue (keeps sync free for xq)
            b1 = consts2.tile([P, NE, MT], f32)
            nc.scalar.dma_start(b1[:], b1_d[:])
            w2 = consts2.tile(w2_shape, fp8)
            nc.scalar.dma_start(w2[:], w2_d[:])
            b2 = consts2.tile([NE, 1], f32)
            nc.scalar.dma_start(b2[:], b2_d[:])
            gb = consts.tile([NE, 1], f32)
            nc.scalar.dma_start(gb[:], gb_d[:])
            ones = consts.tile([NE, 1], f32)
            nc.vector.memset(ones[:], 1.0)
            onesb = consts.tile([NE, 1], bf16)
            nc.vector.memset(onesb[:], 1.0)
            # rec32 rows 1..31 are read (ignored) by stream_shuffle; init once
            rec32 = consts.tile([32, TOK], f32)
            nc.vector.memset(rec32[:], 1.0)

            # w1 chunks stream on the gpsimd queue (own DMA ring)
            def w1_dma(n, mt):
                w1c = wpool.tile([P, KT2, P, 2], fp8)
                nc.gpsimd.dma_start(w1c[:], w1_d[n, mt, :, :, :, :])
                return w1c

            expl = consts.tile([NE, TOK], f32)
            expl_n = consts.tile([NE, TOK], f32)
            recb = consts.tile([32, TOK], f32)
            e_ps_all = ps_e.tile([32, TOK], f32)
            h2s = []          # live h2 pair tiles, expert-major
            e_first = [True]  # first pass of the e_ps_all group

            def h_chain(n, mt, w1c, also_gl=None):
                """One 24-pass w1 chain (optionally interleaved with gating);
                then the h2 requant activation."""
                h_ps = ps_h.tile([P, TOK], f32)
                for k2 in range(KT2):
                    if also_gl is not None:
                        nc.tensor.matmul(
                            also_gl[:], gw[:, k2, :, :], xq[:, k2, :, :],
                            start=(k2 == 0), stop=(k2 == KT2 - 1), perf_mode=DR,
                            skip_group_check=True,
                        )
                    nc.tensor.matmul(
                        h_ps[:], w1c[:, k2, :, :], xq[:, k2, :, :],
                        start=(k2 == 0), stop=(k2 == KT2 - 1), perf_mode=SWI,
                        skip_group_check=True,
                    )
                if mt % 2 == 0:
                    h2 = hpool.tile([P, 2, TOK], fp8)
                    h2s.append(h2)
                # h2[:, mt%2, :] = fp8(16 * relu(h_ps/128 + b1)); b1 is
                # pre-scaled x16 on host so bias applies after the scale
                nc.scalar.activation(
                    h2s[-1][:, mt % 2, :], h_ps[:], AF.Relu,
                    bias=b1[:, n, mt:mt + 1], scale=H_SCALE / X_SCALE,
                )

            def e_dot(n, pair, last=False):
                """One deferred e-dot pass: += (16h).(64 w2) for one
                mid-chunk pair of expert n, into psum row n of e_ps_all."""
                nc.tensor.matmul(
                    e_ps_all[:], w2[:, n, pair, :, :], h2s[2 * n + pair][:, :, :],
                    start=e_first[0], stop=last,
                    perf_mode=(SWI if swi_e else DR), skip_group_check=True,
                )
                e_first[0] = False

            # ---- gating chain first: needs only gw (98KB) + xq chunks, so
            # the PE starts before any w1 chunk lands; w1 e0 chunks stream
            # during it. (interleave=True merges it into e0mt0 instead.) ----
            gl = ps_g.tile([NE, TOK], f32)
            w1c = w1_dma(0, 0)
            w1n = w1_dma(0, 1)
            if not interleave:
                for k2 in range(KT2):
                    nc.tensor.matmul(
                        gl[:], gw[:, k2, :, :], xq[:, k2, :, :],
                        start=(k2 == 0), stop=(k2 == KT2 - 1), perf_mode=DR,
                        skip_group_check=True,
                    )
            h_chain(0, 0, w1c, also_gl=gl if interleave else None)
            # expl[e, t] = exp(gl/128 + gb)
            nc.scalar.activation(
                expl[:], gl[:], AF.Exp, bias=gb[:], scale=1.0 / X_SCALE
            )
            w1c, w1n = w1n, w1_dma(0, 2)
            h_chain(0, 1, w1c)
            w1c, w1n = w1n, w1_dma(0, 3)
            h_chain(0, 2, w1c)
            w1c, w1n = w1n, w1_dma(1, 0)
            h_chain(0, 3, w1c)

            # gate normalization on DVE, off the critical path: rec = 1/den
            # broadcast across partitions via stream_shuffle (den matmul is
            # deferred to the stream tail so the mid-stream stays pure-SWI;
            # a non-SWI pass after an SWI pass costs ~163 ns extra)
            den = ps_g.tile([1, TOK], f32)

            # ---- experts 1..15: pure SWI w1 chains ----
            for n in range(1, NE):
                w1c, w1n = w1n, w1_dma(n, 1)
                h_chain(n, 0, w1c)
                w1c, w1n = w1n, w1_dma(n, 2)
                h_chain(n, 1, w1c)
                w1c, w1n = w1n, w1_dma(n, 3)
                h_chain(n, 2, w1c)
                w1c, w1n = w1n, (w1_dma(n + 1, 0) if n < NE - 1 else None)
                h_chain(n, 3, w1c)

            # ---- stream tail: den, then all 32 e-dots back-to-back (one
            # SWI->DR mode switch total; their h2 inputs are long ready) ----
            nc.tensor.matmul(den[:], ones[:], expl[:], start=True, stop=True,
                             skip_group_check=True)
            nc.vector.reciprocal(rec32[0:1, :], den[:])
            nc.vector.stream_shuffle(recb[:], rec32[:], mask=[0] * 32)
            nc.vector.tensor_mul(expl_n[:], expl[:], recb[0:NE, :])
            for n in range(NE):
                e_dot(n, 0)
                e_dot(n, 1, last=(n == NE - 1))

            # ---- epilogue ----
            # m[n, t] = (e_ps_all[n, t] + 1024*b2[n]) * g_norm[n, t]
            # (bf16: |m| <~ 512, rel err 0.4% -> out err ~2e-4, negligible)
            m = small.tile([NE, TOK], bf16)
            nc.vector.scalar_tensor_tensor(
                m[:], e_ps_all[0:NE, :], b2[:], expl_n[:], ALU.add, ALU.mult
            )
            # u[t] = sum_n m[n, t]
            u_ps = ps_u.tile([1, TOK], f32)
            nc.tensor.matmul(u_ps[:], onesb[:], m[:], start=True, stop=True,
                             skip_group_check=True)
            # out = sigmoid(u / 1024). Sigmoid's ACT table-set differs from
            # the exp/relu set (reload = ~2.7us on the critical tail), but
            # tanh shares exp's set: sigmoid(z) = 0.5 + 0.5*tanh(z/2); the
            # affine 0.5(1+y) is applied host-side during unsharding.
            o = small.tile([1, TOK], f32)
            if tanh_out:
                nc.scalar.activation(
                    o[:], u_ps[:], AF.Tanh, scale=0.5 / E_SCALE
                )
            else:
                nc.scalar.activation(
                    o[:], u_ps[:], AF.Sigmoid, scale=1.0 / E_SCALE
                )
            nc.scalar.dma_start(out_d[:], o[:])

    nc.compile()
    return nc


def _prep_inputs(x, w1, b1, w2, b2, gw, gb):
    """Host-side shard + layout prep. Returns per-core in_maps."""
    fp8np = ml_dtypes.float8_e4m3

    xf = np.ascontiguousarray(np.asarray(x, np.float32)).reshape(T, D)
    # xq[core][p, k2, c, t] = xf[core*TOK + t, k2*256 + c*128 + p]
    xqp = (
        xf.reshape(NCORES, TOK, KT2, 2, P).transpose(0, 4, 2, 3, 1).astype(fp8np)
    )
    # SwInterleave stationary layout: per partition the free bytes are
    # [A_{M-1}, B_{M-1}, ..., A_0, B_0] where A/B are the two 128-row halves
    # (c=0/1) and columns (mid) are stored reversed.
    # w1p[n, mt, p, k2, j, c] = 128 * w1[n, k2*256 + c*128 + p, mt*128 + (127-j)]
    w1s = (np.asarray(w1, np.float32) * X_SCALE).reshape(NE, KT2, 2, P, MT, P)
    w1p = np.ascontiguousarray(
        w1s[..., ::-1].transpose(0, 4, 3, 1, 5, 2).astype(fp8np)
    )
    # gwp[p, k2, c, e] = 128 * gw[k2*256 + c*128 + p, e]  (plain DoubleRow)
    gws = (np.asarray(gw, np.float32) * X_SCALE).reshape(KT2, 2, P, NE)
    gwp = np.ascontiguousarray(gws.transpose(2, 0, 1, 3).astype(fp8np))
    # b1 pre-scaled x16 (applied after the H_SCALE/X_SCALE activation scale)
    b1p = np.ascontiguousarray(
        (np.asarray(b1, np.float32) * H_SCALE)
        .reshape(NE, MT, P)
        .transpose(2, 0, 1)
    )
    # w2 fp8 pairs, zero-padded to 32 stationary cols with expert n's live
    # column landing on psum row n. SWI layout [p, n, mtp, j, c] stores
    # output row (31-j) at position j (cols reversed, pair minor); DR
    # layout [p, n, mtp, c, j] stores row j at position j.
    w2src = (
        (np.asarray(w2, np.float32) * W2_SCALE)
        .reshape(NE, 2, 2, P)
        .transpose(3, 0, 1, 2)
        .astype(fp8np)
    )
    if E_SWI:
        w2p = np.zeros((P, NE, 2, 32, 2), fp8np)
        for n in range(NE):
            w2p[:, n, :, 31 - n, :] = w2src[:, n]
    else:
        w2p = np.zeros((P, NE, 2, 2, 32), fp8np)
        for n in range(NE):
            w2p[:, n, :, :, n] = w2src[:, n]
    b2p = np.asarray(b2, np.float32).reshape(NE, 1) * E_SCALE
    gbp = np.asarray(gb, np.float32).reshape(NE, 1)

    in_maps = []
    for c in range(NCORES):
        in_maps.append(
            {
                "xq": np.ascontiguousarray(xqp[c]),
                "w1": w1p,
                "gw": gwp,
                "b1": b1p,
                "w2": w2p,
                "b2": b2p,
                "gb": gbp,
            }
        )
    return in_maps


def kernel(x, w1, b1, w2, b2, gw, gb):
    from concourse import bass_utils

    if "nc" not in _CACHE:
        _CACHE["nc"] = _build()
    nc = _CACHE["nc"]
    in_maps = _prep_inputs(x, w1, b1, w2, b2, gw, gb)
    res = bass_utils.run_bass_kernel_spmd(nc, in_maps, core_ids=list(range(NCORES)))
    out = np.concatenate([r["out"].reshape(TOK) for r in res.results])
    # device returns tanh(u/2048); sigmoid(u/1024) = 0.5*(1 + tanh)
    out = 0.5 * (1.0 + out)
    return out.reshape(B, NW).astype(np.float32)
